# revision 6
# baseline (speedup 1.0000x reference)
"""Trainium2 Bass kernel for 2-layer GATv2 (nn_GCNAttn_1494648619259), v3.

Contract: kernel(**inputs) takes FULL unsharded inputs (numpy), returns the
FULL output [B, 128, N] float32. 8 NeuronCores = 2 graphs x 4 dst shards,
one launch per GNN layer.  Self-contained.

v3 layout: nodes sorted by in-degree (desc) and packed 128 consecutive
ranks per tile (157 tiles); the edges of dst-slot d live on PARTITION d,
columns 0..deg-1, padded per unit to T_sched[j] columns.  This kills the
dst-side gather (XR stays resident in SBUF, broadcast along columns), the
one-hot S build, and makes the segment-sum an identity-stationary PSUM
accumulation over columns.

Score algebra: |att| is folded into the tables column-wise (lrelu(k*x) =
k*lrelu(x) for k>0), with columns permuted pos-att-first per head, so
  score_h = sum_{pos cols} lrelu(e~) - sum_{neg cols} lrelu(e~),
two range-reduces instead of a separate att-multiply pass.  bl only
shifts the output by a constant (softmax weights sum to 1), so XL tables
are biasless; (bl+br)*s rides in the XR table.  The device emits raw
[weighted-sums | softmax denominator] per node; the host does the
divide / un-scale / un-permute / +bias epilogue in fp32.

Datapath bf16 (tables, gathers, elementwise, PE) with fp32 scores/softmax/
PSUM; lrelu(0.1) is computed on the vector engine as max(x, 0.1x).
"""
import numpy as np
import ml_dtypes
from contextlib import ExitStack

import concourse.bass as bass
import concourse.mybir as mybir
import concourse.tile as tile
from concourse import bacc
from concourse.bass_utils import run_bass_kernel_spmd

# ---- problem constants ----
H = 2
C = 64
F = 2 * C          # 128
NEG = 0.1
N = 20000
Bn = 2
F_IN = 32
P = 128
NT = (N + P - 1) // P       # 157 tiles
J = (NT + 3) // 4           # 40 units per core
NPAD = NT * P               # 20096
NCORES = 8
BTO = 8                     # units per batched hout store
BTA = 8                     # tiles per batched table load/store

BF = mybir.dt.bfloat16
F32 = mybir.dt.float32
NPBF = ml_dtypes.bfloat16

USE_PRELU = False   # Lrelu ignores alpha on HW; Prelu under test; default
                    # to the exact DVE max(x, 0.1x) formulation.


# ======================= host-side planning =======================

def _build_plan_v3(edge_index):
    src0 = edge_index[0].astype(np.int64)
    dst0 = edge_index[1].astype(np.int64)
    loops = np.arange(N, dtype=np.int64)
    src = np.concatenate([src0, loops])
    dst = np.concatenate([dst0, loops])

    deg = np.bincount(dst, minlength=N)          # includes self-loop
    order = np.argsort(-deg, kind="stable")      # rank -> node
    rank = np.empty(N, np.int64)
    rank[order] = np.arange(N)                   # node -> rank

    # incoming edges grouped by dst
    eorder = np.argsort(dst, kind="stable")
    src_by_dst = rank[src[eorder]]               # src ranks, grouped by dst
    starts = np.zeros(N + 1, np.int64)
    starts[1:] = np.cumsum(np.bincount(dst, minlength=N))

    dsort = deg[order]
    tile_max = np.array([int(dsort[t * P:min((t + 1) * P, N)].max())
                         for t in range(NT)])
    T_sched = [max(1, int(tile_max[4 * j:min(4 * j + 4, NT)].max()))
               for j in range(J)]
    SUMT = int(np.sum(T_sched))

    # per-core idx + mask blocks
    cores = []
    for c4 in range(4):
        idx = np.zeros(128 * 16 * 0, np.int64)  # placeholder
        idx_blocks = []
        mask = np.zeros((P, SUMT), np.float32)
        off = 0
        for j in range(J):
            Tj = T_sched[j]
            t = 4 * j + c4
            blk = np.zeros(Tj * P, np.int64)
            if t < NT:
                for d in range(P):
                    r = t * P + d
                    if r >= N:
                        continue
                    node = order[r]
                    ss = src_by_dst[starts[node]:starts[node + 1]]
                    blk[d::P][:len(ss)] = ss
                    mask[d, off:off + len(ss)] = 1.0
            idx_blocks.append(blk)
            off += Tj
        cores.append(dict(idx=np.concatenate(idx_blocks), mask=mask))
    return dict(order=order, rank=rank, T_sched=T_sched, SUMT=SUMT,
                cores=cores)


def _wrap_idx16(idx):
    """[ES] int -> wrapped [128, ES//16] int16: idx i at [i%16, i//16],
    replicated across the 8 GpSimd sub-cores (partitions 16k..16k+15)."""
    es = len(idx)
    a = idx.astype(np.int16).reshape(es // 16, 16).T
    return np.tile(a, (8, 1))


def _layer_consts(W_l, b_l, W_r, b_r, att, bias):
    """att-folded, column-permuted table weights (bf16) + epilogue data."""
    att = np.asarray(att, np.float32)
    perm = np.concatenate([
        h * C + np.concatenate([np.nonzero(att[h] >= 0)[0],
                                np.nonzero(att[h] < 0)[0]])
        for h in range(H)
    ]).astype(np.int64)
    npos = tuple(int((att[h] >= 0).sum()) for h in range(H))
    s = np.abs(att.reshape(-1)[perm]).astype(np.float32)
    s = np.maximum(s, 1e-12)
    Wl = np.asarray(W_l, np.float32)[:, perm] * s[None, :]
    Wr = np.asarray(W_r, np.float32)[:, perm] * s[None, :]
    btil = (np.asarray(b_l, np.float32) + np.asarray(b_r, np.float32))[perm] * s
    return dict(
        Wlp=Wl.astype(NPBF), Wrp=Wr.astype(NPBF),
        btil_b=np.tile(btil[None, :], (P, 1)).astype(NPBF),
        perm=perm, s=s, npos=npos,
        out_add=(np.asarray(b_l, np.float32) + np.asarray(bias, np.float32)),
    )


# ======================= bass kernel build =======================

def _build_layer_program(nc, T_sched, K, npos):
    SUMT = int(np.sum(T_sched))
    TMAX = max(T_sched)
    NIW = 8 * SUMT  # total wrapped idx cols (= 128*SUMT/16)

    featT = nc.dram_tensor("featT", [K, NPAD], BF, kind="ExternalInput").ap()
    featTR = nc.dram_tensor("featTR", [K, J * P], BF,
                            kind="ExternalInput").ap()
    Wlp = nc.dram_tensor("Wlp", [K, F], BF, kind="ExternalInput").ap()
    Wrp = nc.dram_tensor("Wrp", [K, F], BF, kind="ExternalInput").ap()
    btil_in = nc.dram_tensor("btil_b", [P, F], BF, kind="ExternalInput").ap()
    ident_in = nc.dram_tensor("ident", [P, P], BF, kind="ExternalInput").ap()
    mask_in = nc.dram_tensor("mask", [P, SUMT], BF, kind="ExternalInput").ap()
    sidx_in = nc.dram_tensor("sidx", [P, NIW], mybir.dt.int16,
                             kind="ExternalInput").ap()
    hout = nc.dram_tensor("hout", [J * P, F + H], F32,
                          kind="ExternalOutput").ap()

    with tile.TileContext(nc) as tc, ExitStack() as ctx:
        const = ctx.enter_context(tc.tile_pool(name="const", bufs=1))
        dram = ctx.enter_context(tc.tile_pool(name="dram", bufs=1,
                                              space="DRAM"))

        wl_sb = const.tile([K, F], BF)
        nc.sync.dma_start(wl_sb[:], Wlp[:])
        wr_sb = const.tile([K, F], BF)
        nc.sync.dma_start(wr_sb[:], Wrp[:])
        btil_sb = const.tile([P, F], BF)
        nc.sync.dma_start(btil_sb[:], btil_in[:])
        ident_sb = const.tile([P, P], BF)
        nc.sync.dma_start(ident_sb[:], ident_in[:])
        mask_sb = const.tile([P, SUMT], BF)
        nc.sync.dma_start(mask_sb[:], mask_in[:])
        sidx_sb = const.tile([P, NIW], mybir.dt.int16)
        nc.sync.dma_start(sidx_sb[:], sidx_in[:])
        ftr_sb = const.tile([K, J * P], BF)
        nc.sync.dma_start(ftr_sb[:], featTR[:])
        xr_all = const.tile([P, J, F], BF)

        xla = dram.tile([NPAD, F], BF)

        # ---- table phase ----
        nb = (NT + BTA - 1) // BTA
        with tc.tile_pool(name="tab", bufs=3) as tab, \
                tc.tile_pool(name="tps", bufs=3, space="PSUM") as tps:
            for b in range(nb):
                w = min(BTA, NT - b * BTA)
                ft = tab.tile([K, BTA * P], BF, tag="ft")
                nc.sync.dma_start(ft[:, :w * P],
                                  featT[:, b * BTA * P:(b * BTA + w) * P])
                ot = tab.tile([P, BTA, F], BF, tag="ot")
                for q in range(w):
                    pl = tps.tile([P, F], F32, tag="pl", space="PSUM")
                    nc.tensor.matmul(pl[:], ft[:, q * P:(q + 1) * P],
                                     wl_sb[:], start=True, stop=True)
                    nc.scalar.activation(ot[:, q, :], pl[:],
                                         mybir.ActivationFunctionType.Copy)
                nc.sync.dma_start(
                    xla[b * BTA * P:(b * BTA + w) * P, :]
                    .rearrange("(q p) f -> p q f", p=P), ot[:, :w, :])
            for j in range(J):
                pr = tps.tile([P, F], F32, tag="pr", space="PSUM")
                nc.tensor.matmul(pr[:], ftr_sb[:, j * P:(j + 1) * P],
                                 wr_sb[:], start=True, stop=True)
                nc.vector.tensor_tensor(out=xr_all[:, j, :], in0=pr[:],
                                        in1=btil_sb[:],
                                        op=mybir.AluOpType.add)

        # ---- edge phase ----
        gath = ctx.enter_context(tc.tile_pool(name="gath", bufs=4))
        work = ctx.enter_context(tc.tile_pool(name="work", bufs=3))
        ops = ctx.enter_context(tc.tile_pool(name="ops", bufs=4,
                                             space="PSUM"))
        hbp = ctx.enter_context(tc.tile_pool(name="hbp", bufs=2))

        p0, p1 = npos
        hb = None
        off = 0
        ioff = 0
        for j in range(J):
            Tj = T_sched[j]
            ES = Tj * P
            a_g = gath.tile([P, TMAX, F], BF, tag="a")
            nc.gpsimd.dma_gather(
                out_ap=a_g[:, :Tj, :], in_ap=xla[:],
                idxs_ap=sidx_sb[:, ioff:ioff + 8 * Tj],
                num_idxs=ES, num_idxs_reg=ES, elem_size=F,
                single_packet=False, queue_num=j % 4)

            eatt = work.tile([P, TMAX, F], BF, tag="eatt")
            ea = eatt[:, :Tj, :]
            nc.vector.tensor_tensor(
                out=ea, in0=a_g[:, :Tj, :],
                in1=xr_all[:, j, :].unsqueeze(1).to_broadcast([P, Tj, F]),
                op=mybir.AluOpType.add)
            le = work.tile([P, TMAX, F], BF, tag="le")
            ls = le[:, :Tj, :]
            if USE_PRELU:
                nc.scalar.activation(ls, ea,
                                     mybir.ActivationFunctionType.Prelu,
                                     alpha=NEG)
            else:
                # lrelu(x) = max(x, 0.1x), exact on DVE
                nc.vector.scalar_tensor_tensor(
                    out=ls, in0=ea, scalar=NEG, in1=ea,
                    op0=mybir.AluOpType.mult, op1=mybir.AluOpType.max)

            # score_h = sum(pos cols) - sum(neg cols), fp32
            r4 = work.tile([P, TMAX, 4], F32, tag="r4")
            ranges = [(0, p0), (p0, C), (C, C + p1), (C + p1, 2 * C)]
            for i, (a, b2) in enumerate(ranges):
                if b2 > a:
                    nc.vector.tensor_reduce(
                        out=r4[:, :Tj, i], in_=le[:, :Tj, a:b2],
                        axis=mybir.AxisListType.X, op=mybir.AluOpType.add)
                else:
                    nc.gpsimd.memset(r4[:, :Tj, i], 0.0)
            sc = work.tile([P, TMAX, H], F32, tag="sc")
            nc.vector.tensor_tensor(out=sc[:, :Tj, :], in0=r4[:, :Tj, 0::2],
                                    in1=r4[:, :Tj, 1::2],
                                    op=mybir.AluOpType.subtract)
            ex = work.tile([P, TMAX, H], F32, tag="ex")
            nc.scalar.activation(ex[:, :Tj, :], sc[:, :Tj, :],
                                 mybir.ActivationFunctionType.Exp)
            exm = work.tile([P, TMAX, H], F32, tag="exm")
            nc.vector.tensor_tensor(
                out=exm[:, :Tj, :], in0=ex[:, :Tj, :],
                in1=mask_sb[:, off:off + Tj].unsqueeze(2)
                .to_broadcast([P, Tj, H]),
                op=mybir.AluOpType.mult)

            G = work.tile([P, TMAX, F + H], BF, tag="G")
            nc.vector.tensor_tensor(
                out=G[:, :Tj, 0:F].rearrange("p t (h c) -> p t h c", h=H),
                in0=a_g[:, :Tj, :].rearrange("p t (h c) -> p t h c", h=H),
                in1=exm[:, :Tj, :].unsqueeze(3).to_broadcast([P, Tj, H, C]),
                op=mybir.AluOpType.mult)
            nc.vector.tensor_scalar(
                out=G[:, :Tj, F:F + H], in0=exm[:, :Tj, :], scalar1=0.0,
                scalar2=None, op0=mybir.AluOpType.add)

            acc = ops.tile([P, F + H], F32, tag="acc", space="PSUM")
            for tt in range(Tj):
                nc.tensor.matmul(acc[:], ident_sb[:], G[:, tt, :],
                                 start=(tt == 0), stop=(tt == Tj - 1))

            if j % BTO == 0:
                hb = hbp.tile([P, BTO, F + H], F32, tag="hb")
            nc.scalar.activation(hb[:, j % BTO, :], acc[:],
                                 mybir.ActivationFunctionType.Copy)
            if j % BTO == BTO - 1:
                b0 = (j - BTO + 1) * P
                nc.sync.dma_start(
                    hout[b0:b0 + BTO * P, :]
                    .rearrange("(q p) f -> p q f", p=P), hb[:])
            off += Tj
            ioff += 8 * Tj
    return nc


def _compile_layer(T_sched, K, npos):
    nc = bacc.Bacc("TRN2", target_bir_lowering=False, debug=False,
                   enable_asserts=False, num_devices=NCORES,
                   num_swdge_queues=4)
    _build_layer_program(nc, T_sched, K, npos)
    nc.compile()
    return nc


# ======================= top-level kernel =======================

def _make_core_inputs(plan, lc, featT_all, K):
    ident = np.eye(P, dtype=np.float32)
    in_maps = []
    for core in range(NCORES):
        g = core // 4
        c4 = core % 4
        cd = plan["cores"][c4]
        ftg = featT_all[g]
        # featTR: core's own tiles in unit order (dummy -> zeros)
        ftr = np.zeros((K, J * P), np.float32)
        for j in range(J):
            t = 4 * j + c4
            if t < NT:
                ftr[:, j * P:(j + 1) * P] = np.asarray(
                    ftg[:, t * P:(t + 1) * P], np.float32)
        in_maps.append({
            "featT": np.ascontiguousarray(ftg),
            "featTR": ftr.astype(NPBF),
            "Wlp": lc["Wlp"], "Wrp": lc["Wrp"], "btil_b": lc["btil_b"],
            "ident": ident.astype(NPBF),
            "mask": cd["mask"].astype(NPBF),
            "sidx": _wrap_idx16(cd["idx"]),
        })
    return in_maps


def _host_epilogue(res, plan, lc):
    """Per graph: assemble [NPAD, F+H] raw acc, divide/unscale/unpermute,
    add (bl + bias); returns [Bn][NPAD, F] fp32 in rank order."""
    inv_perm = np.empty(F, np.int64)
    inv_perm[lc["perm"]] = np.arange(F)
    outs = []
    for g in range(Bn):
        acc = np.zeros((NPAD, F + H), np.float32)
        for c4 in range(4):
            r = np.asarray(res.results[g * 4 + c4]["hout"], np.float32)
            for j in range(J):
                t = 4 * j + c4
                if t < NT:
                    acc[t * P:(t + 1) * P] = r[j * P:(j + 1) * P]
        den = acc[:, F:F + H] + 1e-30
        hp = acc[:, :F] / lc["s"][None, :]
        hp = hp.reshape(NPAD, H, C) / den[:, :, None]
        h = hp.reshape(NPAD, F)[:, inv_perm] + lc["out_add"][None, :]
        outs.append(h)
    return outs


_RESULTS_LOG = {}


def kernel(x, edge_index, Wl1, bl1, Wr1, br1, att1, bias1,
           Wl2, bl2, Wr2, br2, att2, bias2):
    x = np.asarray(x, np.float32)
    edge_index = np.asarray(edge_index)
    plan = _build_plan_v3(edge_index)
    lc1 = _layer_consts(Wl1, bl1, Wr1, br1, att1, bias1)
    lc2 = _layer_consts(Wl2, bl2, Wr2, br2, att2, bias2)

    # layer 1 features in rank order
    featT1 = []
    for g in range(Bn):
        ft = np.zeros((F_IN, NPAD), np.float32)
        ft[:, :N] = x[g][:, plan["order"]]
        # scatter: column rank r holds node order[r]
        featT1.append(ft.astype(NPBF))

    nc1 = _compile_layer(plan["T_sched"], F_IN, lc1["npos"])
    maps1 = _make_core_inputs(plan, lc1, featT1, F_IN)
    res1 = run_bass_kernel_spmd(nc1, maps1, list(range(NCORES)))
    _RESULTS_LOG["l1"] = res1

    h1 = _host_epilogue(res1, plan, lc1)     # [NPAD, F] rank order
    featT2 = []
    for g in range(Bn):
        hh = h1[g].copy()
        hh[N:] = 0.0
        featT2.append(np.ascontiguousarray(hh.T).astype(NPBF))

    nc2 = _compile_layer(plan["T_sched"], F, lc2["npos"])
    maps2 = _make_core_inputs(plan, lc2, featT2, F)
    res2 = run_bass_kernel_spmd(nc2, maps2, list(range(NCORES)))
    _RESULTS_LOG["l2"] = res2

    h2 = _host_epilogue(res2, plan, lc2)
    out = np.zeros((Bn, F, N), np.float32)
    for g in range(Bn):
        out[g] = h2[g][plan["rank"], :].T
    return out


# revision 10
# speedup vs baseline: 1.3554x; 1.3554x over previous
"""Trainium2 Bass kernel for 2-layer GATv2 (nn_GCNAttn_1494648619259), v3.

Contract: kernel(**inputs) takes FULL unsharded inputs (numpy), returns the
FULL output [B, 128, N] float32. 8 NeuronCores = 2 graphs x 4 dst shards,
one launch per GNN layer.  Self-contained.

v3 layout: nodes sorted by in-degree (desc) and packed 128 consecutive
ranks per tile (157 tiles); the edges of dst-slot d live on PARTITION d,
columns 0..deg-1, padded per unit to T_sched[j] columns.  This kills the
dst-side gather (XR stays resident in SBUF, broadcast along columns), the
one-hot S build, and makes the segment-sum an identity-stationary PSUM
accumulation over columns.

Score algebra: |att| is folded into the tables column-wise (lrelu(k*x) =
k*lrelu(x) for k>0), with columns permuted pos-att-first per head, so
  score_h = sum_{pos cols} lrelu(e~) - sum_{neg cols} lrelu(e~),
two range-reduces instead of a separate att-multiply pass.  bl only
shifts the output by a constant (softmax weights sum to 1), so XL tables
are biasless; (bl+br)*s rides in the XR table.  The device emits raw
[weighted-sums | softmax denominator] per node; the host does the
divide / un-scale / un-permute / +bias epilogue in fp32.

Datapath bf16 (tables, gathers, elementwise, PE) with fp32 scores/softmax/
PSUM; lrelu(0.1) is computed on the vector engine as max(x, 0.1x).
"""
import numpy as np
import ml_dtypes
from contextlib import ExitStack

import concourse.bass as bass
import concourse.mybir as mybir
import concourse.tile as tile
from concourse import bacc
from concourse.bass_utils import run_bass_kernel_spmd

# ---- problem constants ----
H = 2
C = 64
F = 2 * C          # 128
NEG = 0.1
N = 20000
Bn = 2
F_IN = 32
P = 128
NT = (N + P - 1) // P       # 157 tiles
J = (NT + 3) // 4           # 40 units per core
NPAD = NT * P               # 20096
NCORES = 8
BTO = 8                     # units per batched hout store
BTA = 8                     # tiles per batched table load/store

BF = mybir.dt.bfloat16
F32 = mybir.dt.float32
NPBF = ml_dtypes.bfloat16

USE_PRELU = True    # Prelu honors alpha on HW (verified); Lrelu ignores it.


# ======================= host-side planning =======================

def _build_plan_v3(edge_index):
    src0 = edge_index[0].astype(np.int64)
    dst0 = edge_index[1].astype(np.int64)
    loops = np.arange(N, dtype=np.int64)
    src = np.concatenate([src0, loops])
    dst = np.concatenate([dst0, loops])

    deg = np.bincount(dst, minlength=N)          # includes self-loop
    order = np.argsort(-deg, kind="stable")      # rank -> node
    rank = np.empty(N, np.int64)
    rank[order] = np.arange(N)                   # node -> rank

    # incoming edges grouped by dst
    eorder = np.argsort(dst, kind="stable")
    src_by_dst = rank[src[eorder]]               # src ranks, grouped by dst
    starts = np.zeros(N + 1, np.int64)
    starts[1:] = np.cumsum(np.bincount(dst, minlength=N))

    dsort = deg[order]
    tile_max = np.array([int(dsort[t * P:min((t + 1) * P, N)].max())
                         for t in range(NT)])
    T_sched = [max(1, int(tile_max[4 * j:min(4 * j + 4, NT)].max()))
               for j in range(J)]
    SUMT = int(np.sum(T_sched))

    # per-core idx + mask blocks
    cores = []
    for c4 in range(4):
        idx = np.zeros(128 * 16 * 0, np.int64)  # placeholder
        idx_blocks = []
        mask = np.zeros((P, SUMT), np.float32)
        off = 0
        for j in range(J):
            Tj = T_sched[j]
            t = 4 * j + c4
            blk = np.zeros(Tj * P, np.int64)
            if t < NT:
                for d in range(P):
                    r = t * P + d
                    if r >= N:
                        continue
                    node = order[r]
                    ss = src_by_dst[starts[node]:starts[node + 1]]
                    blk[d::P][:len(ss)] = ss
                    mask[d, off:off + len(ss)] = 1.0
            idx_blocks.append(blk)
            off += Tj
        cores.append(dict(idx=np.concatenate(idx_blocks), mask=mask))
    return dict(order=order, rank=rank, T_sched=T_sched, SUMT=SUMT,
                cores=cores)


def _wrap_idx16(idx):
    """[ES] int -> wrapped [128, ES//16] int16: idx i at [i%16, i//16],
    replicated across the 8 GpSimd sub-cores (partitions 16k..16k+15)."""
    es = len(idx)
    a = idx.astype(np.int16).reshape(es // 16, 16).T
    return np.tile(a, (8, 1))


def _layer_consts(W_l, b_l, W_r, b_r, att, bias):
    """att-folded, column-permuted table weights (bf16) + epilogue data."""
    att = np.asarray(att, np.float32)
    perm = np.concatenate([
        h * C + np.concatenate([np.nonzero(att[h] >= 0)[0],
                                np.nonzero(att[h] < 0)[0]])
        for h in range(H)
    ]).astype(np.int64)
    npos = tuple(int((att[h] >= 0).sum()) for h in range(H))
    s = np.abs(att.reshape(-1)[perm]).astype(np.float32)
    s = np.maximum(s, 1e-12)
    Wl = np.asarray(W_l, np.float32)[:, perm] * s[None, :]
    Wr = np.asarray(W_r, np.float32)[:, perm] * s[None, :]
    btil = (np.asarray(b_l, np.float32) + np.asarray(b_r, np.float32))[perm] * s
    return dict(
        Wlp=Wl.astype(NPBF), Wrp=Wr.astype(NPBF),
        btil_b=np.tile(btil[None, :], (P, 1)).astype(NPBF),
        perm=perm, s=s, npos=npos,
        out_add=(np.asarray(b_l, np.float32) + np.asarray(bias, np.float32)),
    )


# ======================= bass kernel build =======================

def _build_layer_program(nc, T_sched, K, npos):
    SUMT = int(np.sum(T_sched))
    TMAX = max(T_sched)
    NIW = 8 * SUMT  # total wrapped idx cols (= 128*SUMT/16)

    featT = nc.dram_tensor("featT", [K, NPAD], BF, kind="ExternalInput").ap()
    featTR = nc.dram_tensor("featTR", [K, J * P], BF,
                            kind="ExternalInput").ap()
    Wlp = nc.dram_tensor("Wlp", [K, F], BF, kind="ExternalInput").ap()
    Wrp = nc.dram_tensor("Wrp", [K, F], BF, kind="ExternalInput").ap()
    btil_in = nc.dram_tensor("btil_b", [P, F], BF, kind="ExternalInput").ap()
    ident_in = nc.dram_tensor("ident", [P, P], BF, kind="ExternalInput").ap()
    mask_in = nc.dram_tensor("mask", [P, SUMT], BF, kind="ExternalInput").ap()
    sidx_in = nc.dram_tensor("sidx", [P, NIW], mybir.dt.int16,
                             kind="ExternalInput").ap()
    hout = nc.dram_tensor("hout", [J * P, F + H], F32,
                          kind="ExternalOutput").ap()

    with tile.TileContext(nc) as tc, ExitStack() as ctx:
        const = ctx.enter_context(tc.tile_pool(name="const", bufs=1))
        dram = ctx.enter_context(tc.tile_pool(name="dram", bufs=1,
                                              space="DRAM"))

        wl_sb = const.tile([K, F], BF)
        nc.sync.dma_start(wl_sb[:], Wlp[:])
        wr_sb = const.tile([K, F], BF)
        nc.sync.dma_start(wr_sb[:], Wrp[:])
        btil_sb = const.tile([P, F], BF)
        nc.sync.dma_start(btil_sb[:], btil_in[:])
        ident_sb = const.tile([P, P], BF)
        nc.sync.dma_start(ident_sb[:], ident_in[:])
        mask_sb = const.tile([P, SUMT], BF)
        nc.sync.dma_start(mask_sb[:], mask_in[:])
        sidx_sb = const.tile([P, NIW], mybir.dt.int16)
        nc.sync.dma_start(sidx_sb[:], sidx_in[:])
        ftr_sb = const.tile([K, J * P], BF)
        nc.sync.dma_start(ftr_sb[:], featTR[:])
        xr_all = const.tile([P, J, F], BF)

        xla = dram.tile([NPAD, F], BF)

        # ---- table phase ----
        nb = (NT + BTA - 1) // BTA
        with tc.tile_pool(name="tab", bufs=3) as tab, \
                tc.tile_pool(name="tps", bufs=3, space="PSUM") as tps:
            for b in range(nb):
                w = min(BTA, NT - b * BTA)
                ft = tab.tile([K, BTA * P], BF, tag="ft")
                nc.sync.dma_start(ft[:, :w * P],
                                  featT[:, b * BTA * P:(b * BTA + w) * P])
                ot = tab.tile([P, BTA, F], BF, tag="ot")
                for q in range(w):
                    pl = tps.tile([P, F], F32, tag="pl", space="PSUM")
                    nc.tensor.matmul(pl[:], ft[:, q * P:(q + 1) * P],
                                     wl_sb[:], start=True, stop=True)
                    nc.scalar.activation(ot[:, q, :], pl[:],
                                         mybir.ActivationFunctionType.Copy)
                nc.sync.dma_start(
                    xla[b * BTA * P:(b * BTA + w) * P, :]
                    .rearrange("(q p) f -> p q f", p=P), ot[:, :w, :])
            for j in range(J):
                pr = tps.tile([P, F], F32, tag="pr", space="PSUM")
                nc.tensor.matmul(pr[:], ftr_sb[:, j * P:(j + 1) * P],
                                 wr_sb[:], start=True, stop=True)
                nc.vector.tensor_tensor(out=xr_all[:, j, :], in0=pr[:],
                                        in1=btil_sb[:],
                                        op=mybir.AluOpType.add)

        # ---- edge phase ----
        gath = ctx.enter_context(tc.tile_pool(name="gath", bufs=4))
        work = ctx.enter_context(tc.tile_pool(name="work", bufs=3))
        ops = ctx.enter_context(tc.tile_pool(name="ops", bufs=4,
                                             space="PSUM"))
        hbp = ctx.enter_context(tc.tile_pool(name="hbp", bufs=2))

        # Gathers are split in half across rotating SWDGE queues: a full
        # 2-4k-idx gather's descriptors overflow the per-queue ring and
        # block Q7 through the DMA drain; halves fit, so Q7 generation
        # pipelines against the other queues' drains.
        p0, p1 = npos
        hb = None
        off = 0
        ioff = 0
        qn = 0
        for j in range(J):
            Tj = T_sched[j]
            a_g = gath.tile([P, TMAX, F], BF, tag="a")
            t_done = 0
            for Tpart in (Tj - Tj // 2, Tj // 2):
                if Tpart == 0:
                    continue
                nc.gpsimd.dma_gather(
                    out_ap=a_g[:, t_done:t_done + Tpart, :], in_ap=xla[:],
                    idxs_ap=sidx_sb[:, ioff + 8 * t_done:
                                    ioff + 8 * (t_done + Tpart)],
                    num_idxs=Tpart * P, num_idxs_reg=Tpart * P,
                    elem_size=F, single_packet=False, queue_num=qn % 4)
                qn += 1
                t_done += Tpart

            eatt = work.tile([P, TMAX, F], BF, tag="eatt")
            ea = eatt[:, :Tj, :]
            nc.vector.tensor_tensor(
                out=ea, in0=a_g[:, :Tj, :],
                in1=xr_all[:, j, :].unsqueeze(1).to_broadcast([P, Tj, F]),
                op=mybir.AluOpType.add)
            le = work.tile([P, TMAX, F], BF, tag="le")
            ls = le[:, :Tj, :]
            if USE_PRELU:
                nc.scalar.activation(ls, ea,
                                     mybir.ActivationFunctionType.Prelu,
                                     alpha=NEG)
            else:
                # lrelu(x) = max(x, 0.1x), exact on DVE
                nc.vector.scalar_tensor_tensor(
                    out=ls, in0=ea, scalar=NEG, in1=ea,
                    op0=mybir.AluOpType.mult, op1=mybir.AluOpType.max)

            # score_h = sum(pos cols) - sum(neg cols), fp32
            r4 = work.tile([P, TMAX, 4], F32, tag="r4")
            ranges = [(0, p0), (p0, C), (C, C + p1), (C + p1, 2 * C)]
            for i, (a, b2) in enumerate(ranges):
                if b2 > a:
                    nc.vector.tensor_reduce(
                        out=r4[:, :Tj, i], in_=le[:, :Tj, a:b2],
                        axis=mybir.AxisListType.X, op=mybir.AluOpType.add)
                else:
                    nc.gpsimd.memset(r4[:, :Tj, i], 0.0)
            sc = work.tile([P, TMAX, H], F32, tag="sc")
            nc.vector.tensor_tensor(out=sc[:, :Tj, :], in0=r4[:, :Tj, 0::2],
                                    in1=r4[:, :Tj, 1::2],
                                    op=mybir.AluOpType.subtract)
            ex = work.tile([P, TMAX, H], F32, tag="ex")
            nc.scalar.activation(ex[:, :Tj, :], sc[:, :Tj, :],
                                 mybir.ActivationFunctionType.Exp)
            exm = work.tile([P, TMAX, H], F32, tag="exm")
            nc.vector.tensor_tensor(
                out=exm[:, :Tj, :], in0=ex[:, :Tj, :],
                in1=mask_sb[:, off:off + Tj].unsqueeze(2)
                .to_broadcast([P, Tj, H]),
                op=mybir.AluOpType.mult)

            G = work.tile([P, TMAX, F], BF, tag="G")
            nc.vector.tensor_tensor(
                out=G[:, :Tj, :].rearrange("p t (h c) -> p t h c", h=H),
                in0=a_g[:, :Tj, :].rearrange("p t (h c) -> p t h c", h=H),
                in1=exm[:, :Tj, :].unsqueeze(3).to_broadcast([P, Tj, H, C]),
                op=mybir.AluOpType.mult)

            acc = ops.tile([P, F], F32, tag="acc", space="PSUM")
            for tt in range(Tj):
                nc.tensor.matmul(acc[:], ident_sb[:], G[:, tt, :],
                                 start=(tt == 0), stop=(tt == Tj - 1))

            if j % BTO == 0:
                hb = hbp.tile([P, BTO, F + H], F32, tag="hb")
            # denominator: sum exm over columns (middle axis via transposed
            # view; 8B inner stride on a tiny tensor)
            nc.vector.tensor_reduce(
                out=hb[:, j % BTO, F:F + H],
                in_=exm[:, :Tj, :].transpose([0, 2, 1]),
                axis=mybir.AxisListType.X, op=mybir.AluOpType.add)
            nc.scalar.activation(hb[:, j % BTO, 0:F], acc[:],
                                 mybir.ActivationFunctionType.Copy)
            if j % BTO == BTO - 1:
                b0 = (j - BTO + 1) * P
                nc.sync.dma_start(
                    hout[b0:b0 + BTO * P, :]
                    .rearrange("(q p) f -> p q f", p=P), hb[:])
            off += Tj
            ioff += 8 * Tj
    return nc


def _compile_layer(T_sched, K, npos):
    nc = bacc.Bacc("TRN2", target_bir_lowering=False, debug=False,
                   enable_asserts=False, num_devices=NCORES,
                   num_swdge_queues=4)
    _build_layer_program(nc, T_sched, K, npos)
    nc.compile()
    return nc


# ======================= top-level kernel =======================

def _make_core_inputs(plan, lc, featT_all, K):
    ident = np.eye(P, dtype=np.float32)
    in_maps = []
    for core in range(NCORES):
        g = core // 4
        c4 = core % 4
        cd = plan["cores"][c4]
        ftg = featT_all[g]
        # featTR: core's own tiles in unit order (dummy -> zeros)
        ftr = np.zeros((K, J * P), np.float32)
        for j in range(J):
            t = 4 * j + c4
            if t < NT:
                ftr[:, j * P:(j + 1) * P] = np.asarray(
                    ftg[:, t * P:(t + 1) * P], np.float32)
        in_maps.append({
            "featT": np.ascontiguousarray(ftg),
            "featTR": ftr.astype(NPBF),
            "Wlp": lc["Wlp"], "Wrp": lc["Wrp"], "btil_b": lc["btil_b"],
            "ident": ident.astype(NPBF),
            "mask": cd["mask"].astype(NPBF),
            "sidx": _wrap_idx16(cd["idx"]),
        })
    return in_maps


def _host_epilogue(res, plan, lc):
    """Per graph: assemble [NPAD, F+H] raw acc, divide/unscale/unpermute,
    add (bl + bias); returns [Bn][NPAD, F] fp32 in rank order."""
    inv_perm = np.empty(F, np.int64)
    inv_perm[lc["perm"]] = np.arange(F)
    outs = []
    for g in range(Bn):
        acc = np.zeros((NPAD, F + H), np.float32)
        for c4 in range(4):
            r = np.asarray(res.results[g * 4 + c4]["hout"], np.float32)
            for j in range(J):
                t = 4 * j + c4
                if t < NT:
                    acc[t * P:(t + 1) * P] = r[j * P:(j + 1) * P]
        den = acc[:, F:F + H] + 1e-30
        hp = acc[:, :F] / lc["s"][None, :]
        hp = hp.reshape(NPAD, H, C) / den[:, :, None]
        h = hp.reshape(NPAD, F)[:, inv_perm] + lc["out_add"][None, :]
        outs.append(h)
    return outs


_RESULTS_LOG = {}


def kernel(x, edge_index, Wl1, bl1, Wr1, br1, att1, bias1,
           Wl2, bl2, Wr2, br2, att2, bias2):
    x = np.asarray(x, np.float32)
    edge_index = np.asarray(edge_index)
    plan = _build_plan_v3(edge_index)
    lc1 = _layer_consts(Wl1, bl1, Wr1, br1, att1, bias1)
    lc2 = _layer_consts(Wl2, bl2, Wr2, br2, att2, bias2)

    # layer 1 features in rank order
    featT1 = []
    for g in range(Bn):
        ft = np.zeros((F_IN, NPAD), np.float32)
        ft[:, :N] = x[g][:, plan["order"]]
        # scatter: column rank r holds node order[r]
        featT1.append(ft.astype(NPBF))

    nc1 = _compile_layer(plan["T_sched"], F_IN, lc1["npos"])
    maps1 = _make_core_inputs(plan, lc1, featT1, F_IN)
    res1 = run_bass_kernel_spmd(nc1, maps1, list(range(NCORES)))
    _RESULTS_LOG["l1"] = res1

    h1 = _host_epilogue(res1, plan, lc1)     # [NPAD, F] rank order
    featT2 = []
    for g in range(Bn):
        hh = h1[g].copy()
        hh[N:] = 0.0
        featT2.append(np.ascontiguousarray(hh.T).astype(NPBF))

    nc2 = _compile_layer(plan["T_sched"], F, lc2["npos"])
    maps2 = _make_core_inputs(plan, lc2, featT2, F)
    res2 = run_bass_kernel_spmd(nc2, maps2, list(range(NCORES)))
    _RESULTS_LOG["l2"] = res2

    h2 = _host_epilogue(res2, plan, lc2)
    out = np.zeros((Bn, F, N), np.float32)
    for g in range(Bn):
        out[g] = h2[g][plan["rank"], :].T
    return out


# revision 12
# speedup vs baseline: 1.4946x; 1.1027x over previous
"""Trainium2 Bass kernel for 2-layer GATv2 (nn_GCNAttn_1494648619259), v3.

Contract: kernel(**inputs) takes FULL unsharded inputs (numpy), returns the
FULL output [B, 128, N] float32. 8 NeuronCores = 2 graphs x 4 dst shards,
one launch per GNN layer.  Self-contained.

v3 layout: nodes sorted by in-degree (desc) and packed 128 consecutive
ranks per tile (157 tiles); the edges of dst-slot d live on PARTITION d,
columns 0..deg-1, padded per unit to T_sched[j] columns.  This kills the
dst-side gather (XR stays resident in SBUF, broadcast along columns), the
one-hot S build, and makes the segment-sum an identity-stationary PSUM
accumulation over columns.

Score algebra: |att| is folded into the tables column-wise (lrelu(k*x) =
k*lrelu(x) for k>0), with columns permuted pos-att-first per head, so
  score_h = sum_{pos cols} lrelu(e~) - sum_{neg cols} lrelu(e~),
two range-reduces instead of a separate att-multiply pass.  bl only
shifts the output by a constant (softmax weights sum to 1), so XL tables
are biasless; (bl+br)*s rides in the XR table.  The device emits raw
[weighted-sums | softmax denominator] per node; the host does the
divide / un-scale / un-permute / +bias epilogue in fp32.

Datapath bf16 (tables, gathers, elementwise, PE) with fp32 scores/softmax/
PSUM; lrelu(0.1) is computed on the vector engine as max(x, 0.1x).
"""
import numpy as np
import ml_dtypes
from contextlib import ExitStack

import concourse.bass as bass
import concourse.mybir as mybir
import concourse.tile as tile
from concourse import bacc
from concourse.bass_utils import run_bass_kernel_spmd

# ---- problem constants ----
H = 2
C = 64
F = 2 * C          # 128
NEG = 0.1
N = 20000
Bn = 2
F_IN = 32
P = 128
NT = (N + P - 1) // P       # 157 tiles
J = (NT + 3) // 4           # 40 units per core
NPAD = NT * P               # 20096
NCORES = 8
BTO = 8                     # units per batched hout store
BTA = 8                     # tiles per batched table load/store

BF = mybir.dt.bfloat16
F32 = mybir.dt.float32
NPBF = ml_dtypes.bfloat16

USE_PRELU = True    # Prelu honors alpha on HW (verified); Lrelu ignores it.


# ======================= host-side planning =======================

def _build_plan_v3(edge_index):
    src0 = edge_index[0].astype(np.int64)
    dst0 = edge_index[1].astype(np.int64)
    loops = np.arange(N, dtype=np.int64)
    src = np.concatenate([src0, loops])
    dst = np.concatenate([dst0, loops])

    deg = np.bincount(dst, minlength=N)          # includes self-loop
    order = np.argsort(-deg, kind="stable")      # rank -> node
    rank = np.empty(N, np.int64)
    rank[order] = np.arange(N)                   # node -> rank

    # incoming edges grouped by dst
    eorder = np.argsort(dst, kind="stable")
    src_by_dst = rank[src[eorder]]               # src ranks, grouped by dst
    starts = np.zeros(N + 1, np.int64)
    starts[1:] = np.cumsum(np.bincount(dst, minlength=N))

    dsort = deg[order]
    tile_max = np.array([int(dsort[t * P:min((t + 1) * P, N)].max())
                         for t in range(NT)])
    T_sched = [max(1, int(tile_max[4 * j:min(4 * j + 4, NT)].max()))
               for j in range(J)]
    SUMT = int(np.sum(T_sched))

    # per-core idx + mask blocks
    cores = []
    for c4 in range(4):
        idx = np.zeros(128 * 16 * 0, np.int64)  # placeholder
        idx_blocks = []
        mask = np.zeros((P, SUMT), np.float32)
        off = 0
        for j in range(J):
            Tj = T_sched[j]
            t = 4 * j + c4
            blk = np.zeros(Tj * P, np.int64)
            if t < NT:
                for d in range(P):
                    r = t * P + d
                    if r >= N:
                        continue
                    node = order[r]
                    ss = src_by_dst[starts[node]:starts[node + 1]]
                    blk[d::P][:len(ss)] = ss
                    mask[d, off:off + len(ss)] = 1.0
            idx_blocks.append(blk)
            off += Tj
        cores.append(dict(idx=np.concatenate(idx_blocks), mask=mask))
    return dict(order=order, rank=rank, T_sched=T_sched, SUMT=SUMT,
                cores=cores)


def _wrap_idx16(idx):
    """[ES] int -> wrapped [128, ES//16] int16: idx i at [i%16, i//16],
    replicated across the 8 GpSimd sub-cores (partitions 16k..16k+15)."""
    es = len(idx)
    a = idx.astype(np.int16).reshape(es // 16, 16).T
    return np.tile(a, (8, 1))


def _layer_consts(W_l, b_l, W_r, b_r, att, bias):
    """att-folded, column-permuted table weights (bf16) + epilogue data."""
    att = np.asarray(att, np.float32)
    perm = np.concatenate([
        h * C + np.concatenate([np.nonzero(att[h] >= 0)[0],
                                np.nonzero(att[h] < 0)[0]])
        for h in range(H)
    ]).astype(np.int64)
    npos = tuple(int((att[h] >= 0).sum()) for h in range(H))
    s = np.abs(att.reshape(-1)[perm]).astype(np.float32)
    s = np.maximum(s, 1e-12)
    Wl = np.asarray(W_l, np.float32)[:, perm] * s[None, :]
    Wr = np.asarray(W_r, np.float32)[:, perm] * s[None, :]
    btil = (np.asarray(b_l, np.float32) + np.asarray(b_r, np.float32))[perm] * s
    return dict(
        Wlp=Wl.astype(NPBF), Wrp=Wr.astype(NPBF),
        btil_b=np.tile(btil[None, :], (P, 1)).astype(NPBF),
        perm=perm, s=s, npos=npos,
        out_add=(np.asarray(b_l, np.float32) + np.asarray(bias, np.float32)),
    )


# ======================= bass kernel build =======================

def _build_layer_program(nc, T_sched, K, npos):
    SUMT = int(np.sum(T_sched))
    TMAX = max(T_sched)
    NIW = 8 * SUMT  # total wrapped idx cols (= 128*SUMT/16)

    featT = nc.dram_tensor("featT", [K, NPAD], BF, kind="ExternalInput").ap()
    featTR = nc.dram_tensor("featTR", [K, J * P], BF,
                            kind="ExternalInput").ap()
    Wlp = nc.dram_tensor("Wlp", [K, F], BF, kind="ExternalInput").ap()
    Wrp = nc.dram_tensor("Wrp", [K, F], BF, kind="ExternalInput").ap()
    btil_in = nc.dram_tensor("btil_b", [P, F], BF, kind="ExternalInput").ap()
    ident_in = nc.dram_tensor("ident", [P, P], BF, kind="ExternalInput").ap()
    mask_in = nc.dram_tensor("mask", [P, SUMT], BF, kind="ExternalInput").ap()
    sidx_in = nc.dram_tensor("sidx", [P, NIW], mybir.dt.int16,
                             kind="ExternalInput").ap()
    hout = nc.dram_tensor("hout", [J * P, F + H], F32,
                          kind="ExternalOutput").ap()

    with tile.TileContext(nc) as tc, ExitStack() as ctx:
        const = ctx.enter_context(tc.tile_pool(name="const", bufs=1))
        dram = ctx.enter_context(tc.tile_pool(name="dram", bufs=1,
                                              space="DRAM"))

        wl_sb = const.tile([K, F], BF)
        nc.sync.dma_start(wl_sb[:], Wlp[:])
        wr_sb = const.tile([K, F], BF)
        nc.sync.dma_start(wr_sb[:], Wrp[:])
        btil_sb = const.tile([P, F], BF)
        nc.sync.dma_start(btil_sb[:], btil_in[:])
        ident_sb = const.tile([P, P], BF)
        nc.sync.dma_start(ident_sb[:], ident_in[:])
        mask_sb = const.tile([P, SUMT], BF)
        nc.sync.dma_start(mask_sb[:], mask_in[:])
        sidx_sb = const.tile([P, NIW], mybir.dt.int16)
        nc.sync.dma_start(sidx_sb[:], sidx_in[:])
        ftr_sb = const.tile([K, J * P], BF)
        nc.sync.dma_start(ftr_sb[:], featTR[:])
        xr_all = const.tile([P, J, F], BF)

        xla = dram.tile([NPAD, F], BF)

        # ---- table phase ----
        nb = (NT + BTA - 1) // BTA
        with tc.tile_pool(name="tab", bufs=3) as tab, \
                tc.tile_pool(name="tps", bufs=3, space="PSUM") as tps:
            for b in range(nb):
                w = min(BTA, NT - b * BTA)
                ft = tab.tile([K, BTA * P], BF, tag="ft")
                nc.sync.dma_start(ft[:, :w * P],
                                  featT[:, b * BTA * P:(b * BTA + w) * P])
                ot = tab.tile([P, BTA, F], BF, tag="ot")
                for q in range(w):
                    pl = tps.tile([P, F], F32, tag="pl", space="PSUM")
                    nc.tensor.matmul(pl[:], ft[:, q * P:(q + 1) * P],
                                     wl_sb[:], start=True, stop=True)
                    # split PSUM->SBUF copies across ACT and DVE so the
                    # serial table phase is bounded by neither
                    if q % 2 == 0:
                        nc.scalar.activation(
                            ot[:, q, :], pl[:],
                            mybir.ActivationFunctionType.Copy)
                    else:
                        nc.vector.tensor_scalar(
                            out=ot[:, q, :], in0=pl[:], scalar1=0.0,
                            scalar2=None, op0=mybir.AluOpType.add)
                nc.sync.dma_start(
                    xla[b * BTA * P:(b * BTA + w) * P, :]
                    .rearrange("(q p) f -> p q f", p=P), ot[:, :w, :])
            for j in range(J):
                pr = tps.tile([P, F], F32, tag="pr", space="PSUM")
                nc.tensor.matmul(pr[:], ftr_sb[:, j * P:(j + 1) * P],
                                 wr_sb[:], start=True, stop=True)
                nc.vector.tensor_tensor(out=xr_all[:, j, :], in0=pr[:],
                                        in1=btil_sb[:],
                                        op=mybir.AluOpType.add)

        # ---- edge phase ----
        gath = ctx.enter_context(tc.tile_pool(name="gath", bufs=5))
        work = ctx.enter_context(tc.tile_pool(name="work", bufs=4))
        ops = ctx.enter_context(tc.tile_pool(name="ops", bufs=4,
                                             space="PSUM"))
        hbp = ctx.enter_context(tc.tile_pool(name="hbp", bufs=2))

        # Gathers are split in half across rotating SWDGE queues: a full
        # 2-4k-idx gather's descriptors overflow the per-queue ring and
        # block Q7 through the DMA drain; halves fit, so Q7 generation
        # pipelines against the other queues' drains.
        p0, p1 = npos
        hb = None
        off = 0
        ioff = 0
        qn = 0
        for j in range(J):
            Tj = T_sched[j]
            a_g = gath.tile([P, TMAX, F], BF, tag="a")
            t_done = 0
            for Tpart in (Tj - Tj // 2, Tj // 2):
                if Tpart == 0:
                    continue
                nc.gpsimd.dma_gather(
                    out_ap=a_g[:, t_done:t_done + Tpart, :], in_ap=xla[:],
                    idxs_ap=sidx_sb[:, ioff + 8 * t_done:
                                    ioff + 8 * (t_done + Tpart)],
                    num_idxs=Tpart * P, num_idxs_reg=Tpart * P,
                    elem_size=F, single_packet=False, queue_num=qn % 4)
                qn += 1
                t_done += Tpart

            eatt = work.tile([P, TMAX, F], BF, tag="eatt")
            ea = eatt[:, :Tj, :]
            nc.vector.tensor_tensor(
                out=ea, in0=a_g[:, :Tj, :],
                in1=xr_all[:, j, :].unsqueeze(1).to_broadcast([P, Tj, F]),
                op=mybir.AluOpType.add)
            le = work.tile([P, TMAX, F], BF, tag="le")
            ls = le[:, :Tj, :]
            if USE_PRELU:
                nc.scalar.activation(ls, ea,
                                     mybir.ActivationFunctionType.Prelu,
                                     alpha=NEG)
            else:
                # lrelu(x) = max(x, 0.1x), exact on DVE
                nc.vector.scalar_tensor_tensor(
                    out=ls, in0=ea, scalar=NEG, in1=ea,
                    op0=mybir.AluOpType.mult, op1=mybir.AluOpType.max)

            # score_h = sum(pos cols) - sum(neg cols), fp32
            r4 = work.tile([P, TMAX, 4], F32, tag="r4")
            ranges = [(0, p0), (p0, C), (C, C + p1), (C + p1, 2 * C)]
            for i, (a, b2) in enumerate(ranges):
                if b2 > a:
                    nc.vector.tensor_reduce(
                        out=r4[:, :Tj, i], in_=le[:, :Tj, a:b2],
                        axis=mybir.AxisListType.X, op=mybir.AluOpType.add)
                else:
                    nc.gpsimd.memset(r4[:, :Tj, i], 0.0)
            sc = work.tile([P, TMAX, H], F32, tag="sc")
            nc.vector.tensor_tensor(out=sc[:, :Tj, :], in0=r4[:, :Tj, 0::2],
                                    in1=r4[:, :Tj, 1::2],
                                    op=mybir.AluOpType.subtract)
            ex = work.tile([P, TMAX, H], F32, tag="ex")
            nc.scalar.activation(ex[:, :Tj, :], sc[:, :Tj, :],
                                 mybir.ActivationFunctionType.Exp)
            exm = work.tile([P, TMAX, H], F32, tag="exm")
            nc.vector.tensor_tensor(
                out=exm[:, :Tj, :], in0=ex[:, :Tj, :],
                in1=mask_sb[:, off:off + Tj].unsqueeze(2)
                .to_broadcast([P, Tj, H]),
                op=mybir.AluOpType.mult)

            G = work.tile([P, TMAX, F], BF, tag="G")
            nc.vector.tensor_tensor(
                out=G[:, :Tj, :].rearrange("p t (h c) -> p t h c", h=H),
                in0=a_g[:, :Tj, :].rearrange("p t (h c) -> p t h c", h=H),
                in1=exm[:, :Tj, :].unsqueeze(3).to_broadcast([P, Tj, H, C]),
                op=mybir.AluOpType.mult)

            acc = ops.tile([P, F], F32, tag="acc", space="PSUM")
            for tt in range(Tj):
                nc.tensor.matmul(acc[:], ident_sb[:], G[:, tt, :],
                                 start=(tt == 0), stop=(tt == Tj - 1))

            if j % BTO == 0:
                hb = hbp.tile([P, BTO, F + H], F32, tag="hb")
            # denominator: sum exm over columns (middle axis via transposed
            # view; 8B inner stride on a tiny tensor)
            nc.vector.tensor_reduce(
                out=hb[:, j % BTO, F:F + H],
                in_=exm[:, :Tj, :].transpose([0, 2, 1]),
                axis=mybir.AxisListType.X, op=mybir.AluOpType.add)
            nc.scalar.activation(hb[:, j % BTO, 0:F], acc[:],
                                 mybir.ActivationFunctionType.Copy)
            if j % BTO == BTO - 1:
                b0 = (j - BTO + 1) * P
                nc.sync.dma_start(
                    hout[b0:b0 + BTO * P, :]
                    .rearrange("(q p) f -> p q f", p=P), hb[:])
            off += Tj
            ioff += 8 * Tj
    return nc


def _compile_layer(T_sched, K, npos):
    nc = bacc.Bacc("TRN2", target_bir_lowering=False, debug=False,
                   enable_asserts=False, num_devices=NCORES,
                   num_swdge_queues=4)
    _build_layer_program(nc, T_sched, K, npos)
    nc.compile()
    return nc


# ======================= top-level kernel =======================

def _make_core_inputs(plan, lc, featT_all, K):
    ident = np.eye(P, dtype=np.float32)
    in_maps = []
    for core in range(NCORES):
        g = core // 4
        c4 = core % 4
        cd = plan["cores"][c4]
        ftg = featT_all[g]
        # featTR: core's own tiles in unit order (dummy -> zeros)
        ftr = np.zeros((K, J * P), np.float32)
        for j in range(J):
            t = 4 * j + c4
            if t < NT:
                ftr[:, j * P:(j + 1) * P] = np.asarray(
                    ftg[:, t * P:(t + 1) * P], np.float32)
        in_maps.append({
            "featT": np.ascontiguousarray(ftg),
            "featTR": ftr.astype(NPBF),
            "Wlp": lc["Wlp"], "Wrp": lc["Wrp"], "btil_b": lc["btil_b"],
            "ident": ident.astype(NPBF),
            "mask": cd["mask"].astype(NPBF),
            "sidx": _wrap_idx16(cd["idx"]),
        })
    return in_maps


def _host_epilogue(res, plan, lc):
    """Per graph: assemble [NPAD, F+H] raw acc, divide/unscale/unpermute,
    add (bl + bias); returns [Bn][NPAD, F] fp32 in rank order."""
    inv_perm = np.empty(F, np.int64)
    inv_perm[lc["perm"]] = np.arange(F)
    outs = []
    for g in range(Bn):
        acc = np.zeros((NPAD, F + H), np.float32)
        for c4 in range(4):
            r = np.asarray(res.results[g * 4 + c4]["hout"], np.float32)
            for j in range(J):
                t = 4 * j + c4
                if t < NT:
                    acc[t * P:(t + 1) * P] = r[j * P:(j + 1) * P]
        den = acc[:, F:F + H] + 1e-30
        hp = acc[:, :F] / lc["s"][None, :]
        hp = hp.reshape(NPAD, H, C) / den[:, :, None]
        h = hp.reshape(NPAD, F)[:, inv_perm] + lc["out_add"][None, :]
        outs.append(h)
    return outs


_RESULTS_LOG = {}


def kernel(x, edge_index, Wl1, bl1, Wr1, br1, att1, bias1,
           Wl2, bl2, Wr2, br2, att2, bias2):
    x = np.asarray(x, np.float32)
    edge_index = np.asarray(edge_index)
    plan = _build_plan_v3(edge_index)
    lc1 = _layer_consts(Wl1, bl1, Wr1, br1, att1, bias1)
    lc2 = _layer_consts(Wl2, bl2, Wr2, br2, att2, bias2)

    # layer 1 features in rank order
    featT1 = []
    for g in range(Bn):
        ft = np.zeros((F_IN, NPAD), np.float32)
        ft[:, :N] = x[g][:, plan["order"]]
        # scatter: column rank r holds node order[r]
        featT1.append(ft.astype(NPBF))

    nc1 = _compile_layer(plan["T_sched"], F_IN, lc1["npos"])
    maps1 = _make_core_inputs(plan, lc1, featT1, F_IN)
    res1 = run_bass_kernel_spmd(nc1, maps1, list(range(NCORES)))
    _RESULTS_LOG["l1"] = res1

    h1 = _host_epilogue(res1, plan, lc1)     # [NPAD, F] rank order
    featT2 = []
    for g in range(Bn):
        hh = h1[g].copy()
        hh[N:] = 0.0
        featT2.append(np.ascontiguousarray(hh.T).astype(NPBF))

    nc2 = _compile_layer(plan["T_sched"], F, lc2["npos"])
    maps2 = _make_core_inputs(plan, lc2, featT2, F)
    res2 = run_bass_kernel_spmd(nc2, maps2, list(range(NCORES)))
    _RESULTS_LOG["l2"] = res2

    h2 = _host_epilogue(res2, plan, lc2)
    out = np.zeros((Bn, F, N), np.float32)
    for g in range(Bn):
        out[g] = h2[g][plan["rank"], :].T
    return out


# revision 13
# speedup vs baseline: 1.5012x; 1.0044x over previous
"""Trainium2 Bass kernel for 2-layer GATv2 (nn_GCNAttn_1494648619259), v6.

Contract: kernel(**inputs) takes FULL unsharded inputs (numpy), returns the
FULL output [B, 128, N] float32. One launch per GNN layer; 8 NeuronCores.
Self-contained.

v6 sharding: nodes sorted by in-degree, 128 consecutive ranks per tile
(157 tiles) striped over all 8 cores (core c owns tiles t = 8j + c); each
core processes its ~20 tiles for BOTH graphs.  Tables are stored with the
two graphs' rows interleaved ([rank, 2, F] bf16), so ONE 512-byte-element
gather per unit fetches both graphs' source rows — half the SWDGE
descriptor work of per-graph gathers.  Gathers are further split in half
across rotating SWDGE queues so Q7 generation pipelines with DMA drains.

Edge layout: the edges of dst-slot d live on PARTITION d, columns
0..deg-1.  The dst-side transform XR stays resident in SBUF (computed per
unit from featTR) and broadcasts along columns; the segment-sum is an
identity-stationary PSUM accumulation over columns; the softmax
denominator is a tiny transposed reduce of the exp scores.

Pad slots gather a POISON table row (-B in pos-att columns, +B in
neg-att columns, B=1e8) so their scores are <= -0.1B and exp underflows
to exactly 0 — no mask multiply needed.

Score algebra: |att| folded into tables column-wise (pos-att columns
first per head), score_h = sum(pos cols) - sum(neg cols) of
lrelu(e~) (Prelu alpha=0.1 on the scalar engine); bl rides in the final
host bias (softmax weights sum to 1), (bl+br)*s rides in XR.  The device
emits raw [weighted sums | denominator]; the host epilogue does
divide / un-scale / un-permute / +bias in fp32.
"""
import numpy as np
import ml_dtypes
from contextlib import ExitStack

import concourse.bass as bass
import concourse.mybir as mybir
import concourse.tile as tile
from concourse import bacc
from concourse.bass_utils import run_bass_kernel_spmd

# ---- problem constants ----
H = 2
C = 64
F = 2 * C          # 128
NEG = 0.1
N = 20000
Bn = 2
F_IN = 32
P = 128
NT = (N + P - 1) // P       # 157 tiles
J = (NT + 7) // 8           # 20 units per core
NPAD = NT * P               # 20096
NCORES = 8
BTO = 4                     # units per batched hout store
BTA = 8                     # tiles per batched table load/store
POISON = 1.0e8

BF = mybir.dt.bfloat16
F32 = mybir.dt.float32
NPBF = ml_dtypes.bfloat16


# ======================= host-side planning =======================

def _build_plan_v6(edge_index):
    src0 = edge_index[0].astype(np.int64)
    dst0 = edge_index[1].astype(np.int64)
    loops = np.arange(N, dtype=np.int64)
    src = np.concatenate([src0, loops])
    dst = np.concatenate([dst0, loops])

    deg = np.bincount(dst, minlength=N)          # includes self-loop
    order = np.argsort(-deg, kind="stable")      # rank -> node
    rank = np.empty(N, np.int64)
    rank[order] = np.arange(N)                   # node -> rank

    eorder = np.argsort(dst, kind="stable")
    src_by_dst = rank[src[eorder]]               # src ranks grouped by dst
    starts = np.zeros(N + 1, np.int64)
    starts[1:] = np.cumsum(np.bincount(dst, minlength=N))

    dsort = deg[order]
    tile_max = np.array([int(dsort[t * P:min((t + 1) * P, N)].max())
                         for t in range(NT)])
    T_sched = [max(1, int(tile_max[8 * j:min(8 * j + 8, NT)].max()))
               for j in range(J)]
    SUMT = int(np.sum(T_sched))

    # per-core idx blocks; pad slots -> poison element (index NPAD)
    cores = []
    for c in range(NCORES):
        idx_blocks = []
        for j in range(J):
            Tj = T_sched[j]
            t = 8 * j + c
            blk = np.full(Tj * P, NPAD, dtype=np.int64)
            if t < NT:
                for d in range(P):
                    r = t * P + d
                    if r >= N:
                        continue
                    node = order[r]
                    ss = src_by_dst[starts[node]:starts[node + 1]]
                    blk[d::P][:len(ss)] = ss
            idx_blocks.append(blk)
        cores.append(np.concatenate(idx_blocks))
    return dict(order=order, rank=rank, T_sched=T_sched, SUMT=SUMT,
                cores=cores)


def _wrap_idx16(idx):
    es = len(idx)
    a = idx.astype(np.int16).reshape(es // 16, 16).T
    return np.tile(a, (8, 1))


def _layer_consts(W_l, b_l, W_r, b_r, att, bias):
    att = np.asarray(att, np.float32)
    perm = np.concatenate([
        h * C + np.concatenate([np.nonzero(att[h] >= 0)[0],
                                np.nonzero(att[h] < 0)[0]])
        for h in range(H)
    ]).astype(np.int64)
    npos = tuple(int((att[h] >= 0).sum()) for h in range(H))
    s = np.abs(att.reshape(-1)[perm]).astype(np.float32)
    s = np.maximum(s, 1e-12)
    Wl = np.asarray(W_l, np.float32)[:, perm] * s[None, :]
    Wr = np.asarray(W_r, np.float32)[:, perm] * s[None, :]
    btil = (np.asarray(b_l, np.float32) + np.asarray(b_r, np.float32))[perm] * s
    # poison row: -B in pos-att cols, +B in neg-att cols (per head)
    pois = np.empty(F, np.float32)
    for h in range(H):
        pois[h * C:h * C + npos[h]] = -POISON
        pois[h * C + npos[h]:(h + 1) * C] = POISON
    return dict(
        Wlp=Wl.astype(NPBF), Wrp=Wr.astype(NPBF),
        btil_b=np.tile(btil[None, :], (P, 1)).astype(NPBF),
        pois_b=np.tile(np.concatenate([pois, pois])[None, :],
                       (P, 1)).astype(NPBF),
        perm=perm, s=s, npos=npos,
        out_add=(np.asarray(b_l, np.float32) + np.asarray(bias, np.float32)),
    )


# ======================= bass kernel build =======================

def _build_layer_program(nc, T_sched, K, npos):
    SUMT = int(np.sum(T_sched))
    TMAX = max(T_sched)
    NIW = 8 * SUMT
    EF = 2 * F   # interleaved element: both graphs' rows

    featT = nc.dram_tensor("featT", [Bn, K, NPAD], BF,
                           kind="ExternalInput").ap()
    featTR = nc.dram_tensor("featTR", [K, J * Bn * P], BF,
                            kind="ExternalInput").ap()
    Wlp = nc.dram_tensor("Wlp", [K, F], BF, kind="ExternalInput").ap()
    Wrp = nc.dram_tensor("Wrp", [K, F], BF, kind="ExternalInput").ap()
    btil_in = nc.dram_tensor("btil_b", [P, F], BF, kind="ExternalInput").ap()
    pois_in = nc.dram_tensor("pois_b", [P, EF], BF, kind="ExternalInput").ap()
    ident_in = nc.dram_tensor("ident", [P, P], BF, kind="ExternalInput").ap()
    sidx_in = nc.dram_tensor("sidx", [P, NIW], mybir.dt.int16,
                             kind="ExternalInput").ap()
    hout = nc.dram_tensor("hout", [J * P, Bn * (F + H)], F32,
                          kind="ExternalOutput").ap()

    with tile.TileContext(nc) as tc, ExitStack() as ctx:
        const = ctx.enter_context(tc.tile_pool(name="const", bufs=1))
        dram = ctx.enter_context(tc.tile_pool(name="dram", bufs=1,
                                              space="DRAM"))

        wl_sb = const.tile([K, F], BF)
        nc.sync.dma_start(wl_sb[:], Wlp[:])
        wr_sb = const.tile([K, F], BF)
        nc.sync.dma_start(wr_sb[:], Wrp[:])
        btil_sb = const.tile([P, F], BF)
        nc.sync.dma_start(btil_sb[:], btil_in[:])
        pois_sb = const.tile([P, EF], BF)
        nc.sync.dma_start(pois_sb[:], pois_in[:])
        ident_sb = const.tile([P, P], BF)
        nc.sync.dma_start(ident_sb[:], ident_in[:])
        sidx_sb = const.tile([P, NIW], mybir.dt.int16)
        nc.sync.dma_start(sidx_sb[:], sidx_in[:])
        ftr_sb = const.tile([K, J * Bn * P], BF)
        nc.sync.dma_start(ftr_sb[:], featTR[:])
        xr_all = const.tile([P, J, Bn, F], BF)

        # interleaved table + one poison block at rows NPAD..NPAD+127
        xla = dram.tile([NPAD + P, EF], BF)
        nc.sync.dma_start(xla[NPAD:NPAD + P, :], pois_sb[:])

        # ---- table phase: XL for all 157 tiles x both graphs ----
        nb = (NT + BTA - 1) // BTA
        with tc.tile_pool(name="tab", bufs=3) as tab, \
                tc.tile_pool(name="tps", bufs=3, space="PSUM") as tps:
            for b in range(nb):
                w = min(BTA, NT - b * BTA)
                ot = tab.tile([P, BTA, EF], BF, tag="ot")
                for g in range(Bn):
                    ft = tab.tile([K, BTA * P], BF, tag=f"ft{g}")
                    nc.sync.dma_start(
                        ft[:, :w * P],
                        featT[g, :, b * BTA * P:(b * BTA + w) * P])
                    for q in range(w):
                        pl = tps.tile([P, F], F32, tag="pl", space="PSUM")
                        nc.tensor.matmul(pl[:], ft[:, q * P:(q + 1) * P],
                                         wl_sb[:], start=True, stop=True)
                        if (2 * q + g) % 2 == 0:
                            nc.scalar.activation(
                                ot[:, q, g * F:(g + 1) * F], pl[:],
                                mybir.ActivationFunctionType.Copy)
                        else:
                            nc.vector.tensor_scalar(
                                out=ot[:, q, g * F:(g + 1) * F], in0=pl[:],
                                scalar1=0.0, scalar2=None,
                                op0=mybir.AluOpType.add)
                nc.sync.dma_start(
                    xla[b * BTA * P:(b * BTA + w) * P, :]
                    .rearrange("(q p) e -> p q e", p=P), ot[:, :w, :])
            # XR for own tiles, both graphs, resident in SBUF
            for j in range(J):
                for g in range(Bn):
                    pr = tps.tile([P, F], F32, tag="pr", space="PSUM")
                    s0 = (j * Bn + g) * P
                    nc.tensor.matmul(pr[:], ftr_sb[:, s0:s0 + P],
                                     wr_sb[:], start=True, stop=True)
                    nc.vector.tensor_tensor(out=xr_all[:, j, g, :],
                                            in0=pr[:], in1=btil_sb[:],
                                            op=mybir.AluOpType.add)

        # ---- edge phase ----
        gath = ctx.enter_context(tc.tile_pool(name="gath", bufs=3))
        work = ctx.enter_context(tc.tile_pool(name="work", bufs=3))
        ops = ctx.enter_context(tc.tile_pool(name="ops", bufs=4,
                                             space="PSUM"))
        hbp = ctx.enter_context(tc.tile_pool(name="hbp", bufs=2))

        p0, p1 = npos
        ranges = [(0, p0), (p0, C), (C, C + p1), (C + p1, 2 * C)]
        hb = None
        ioff = 0
        qn = 0
        for j in range(J):
            Tj = T_sched[j]
            a_g = gath.tile([P, TMAX, EF], BF, tag="a")
            t_done = 0
            for Tpart in (Tj - Tj // 2, Tj // 2):
                if Tpart == 0:
                    continue
                nc.gpsimd.dma_gather(
                    out_ap=a_g[:, t_done:t_done + Tpart, :], in_ap=xla[:],
                    idxs_ap=sidx_sb[:, ioff + 8 * t_done:
                                    ioff + 8 * (t_done + Tpart)],
                    num_idxs=Tpart * P, num_idxs_reg=Tpart * P,
                    elem_size=EF, single_packet=False, queue_num=qn % 4)
                qn += 1
                t_done += Tpart

            if j % BTO == 0:
                hb = hbp.tile([P, BTO, Bn, F + H], F32, tag="hb")
            for g in range(Bn):
                ag = a_g[:, :Tj, g * F:(g + 1) * F]
                eatt = work.tile([P, TMAX, F], BF, tag="eatt")
                ea = eatt[:, :Tj, :]
                nc.vector.tensor_tensor(
                    out=ea, in0=ag,
                    in1=xr_all[:, j, g, :].unsqueeze(1)
                    .to_broadcast([P, Tj, F]),
                    op=mybir.AluOpType.add)
                le = work.tile([P, TMAX, F], BF, tag="le")
                ls = le[:, :Tj, :]
                nc.scalar.activation(ls, ea,
                                     mybir.ActivationFunctionType.Prelu,
                                     alpha=NEG)
                r4 = work.tile([P, TMAX, 4], F32, tag="r4")
                for i, (a, b2) in enumerate(ranges):
                    if b2 > a:
                        nc.vector.tensor_reduce(
                            out=r4[:, :Tj, i], in_=le[:, :Tj, a:b2],
                            axis=mybir.AxisListType.X,
                            op=mybir.AluOpType.add)
                    else:
                        nc.gpsimd.memset(r4[:, :Tj, i], 0.0)
                sc = work.tile([P, TMAX, H], F32, tag="sc")
                nc.vector.tensor_tensor(
                    out=sc[:, :Tj, :], in0=r4[:, :Tj, 0::2],
                    in1=r4[:, :Tj, 1::2], op=mybir.AluOpType.subtract)
                ex = work.tile([P, TMAX, H], F32, tag="ex")
                nc.scalar.activation(ex[:, :Tj, :], sc[:, :Tj, :],
                                     mybir.ActivationFunctionType.Exp)

                G = work.tile([P, TMAX, F], BF, tag="G")
                nc.vector.tensor_tensor(
                    out=G[:, :Tj, :].rearrange("p t (h c) -> p t h c", h=H),
                    in0=ag.rearrange("p t (h c) -> p t h c", h=H),
                    in1=ex[:, :Tj, :].unsqueeze(3)
                    .to_broadcast([P, Tj, H, C]),
                    op=mybir.AluOpType.mult)

                acc = ops.tile([P, F], F32, tag="acc", space="PSUM")
                for tt in range(Tj):
                    nc.tensor.matmul(acc[:], ident_sb[:], G[:, tt, :],
                                     start=(tt == 0), stop=(tt == Tj - 1))

                nc.vector.tensor_reduce(
                    out=hb[:, j % BTO, g, F:F + H],
                    in_=ex[:, :Tj, :].transpose([0, 2, 1]),
                    axis=mybir.AxisListType.X, op=mybir.AluOpType.add)
                nc.scalar.activation(hb[:, j % BTO, g, 0:F], acc[:],
                                     mybir.ActivationFunctionType.Copy)
            if j % BTO == BTO - 1:
                b0 = (j - BTO + 1) * P
                nc.sync.dma_start(
                    hout[b0:b0 + BTO * P, :]
                    .rearrange("(q p) (g e) -> p q g e", p=P, g=Bn), hb[:])
            ioff += 8 * Tj
    return nc


def _compile_layer(T_sched, K, npos):
    nc = bacc.Bacc("TRN2", target_bir_lowering=False, debug=False,
                   enable_asserts=False, num_devices=NCORES,
                   num_swdge_queues=4)
    _build_layer_program(nc, T_sched, K, npos)
    nc.compile()
    return nc


# ======================= top-level kernel =======================

def _make_core_inputs(plan, lc, featT_both, K):
    """featT_both: [Bn, K, NPAD] bf16."""
    ident = np.eye(P, dtype=np.float32)
    in_maps = []
    for core in range(NCORES):
        ftr = np.zeros((K, J * Bn * P), np.float32)
        for j in range(J):
            t = 8 * j + core
            if t < NT:
                for g in range(Bn):
                    ftr[:, (j * Bn + g) * P:(j * Bn + g + 1) * P] = \
                        np.asarray(featT_both[g, :, t * P:(t + 1) * P],
                                   np.float32)
        in_maps.append({
            "featT": featT_both,
            "featTR": ftr.astype(NPBF),
            "Wlp": lc["Wlp"], "Wrp": lc["Wrp"], "btil_b": lc["btil_b"],
            "pois_b": lc["pois_b"],
            "ident": ident.astype(NPBF),
            "sidx": _wrap_idx16(plan["cores"][core]),
        })
    return in_maps


def _host_epilogue(res, plan, lc):
    inv_perm = np.empty(F, np.int64)
    inv_perm[lc["perm"]] = np.arange(F)
    outs = []
    for g in range(Bn):
        acc = np.zeros((NPAD, F + H), np.float32)
        for core in range(NCORES):
            r = np.asarray(res.results[core]["hout"], np.float32).reshape(
                J * P, Bn, F + H)
            for j in range(J):
                t = 8 * j + core
                if t < NT:
                    acc[t * P:(t + 1) * P] = r[j * P:(j + 1) * P, g]
        den = acc[:, F:F + H] + 1e-30
        hp = acc[:, :F] / lc["s"][None, :]
        hp = hp.reshape(NPAD, H, C) / den[:, :, None]
        h = hp.reshape(NPAD, F)[:, inv_perm] + lc["out_add"][None, :]
        outs.append(h)
    return outs


_RESULTS_LOG = {}


def kernel(x, edge_index, Wl1, bl1, Wr1, br1, att1, bias1,
           Wl2, bl2, Wr2, br2, att2, bias2):
    x = np.asarray(x, np.float32)
    edge_index = np.asarray(edge_index)
    plan = _build_plan_v6(edge_index)
    lc1 = _layer_consts(Wl1, bl1, Wr1, br1, att1, bias1)
    lc2 = _layer_consts(Wl2, bl2, Wr2, br2, att2, bias2)

    featT1 = np.zeros((Bn, F_IN, NPAD), np.float32)
    for g in range(Bn):
        featT1[g, :, :N] = x[g][:, plan["order"]]
    featT1 = featT1.astype(NPBF)

    nc1 = _compile_layer(plan["T_sched"], F_IN, lc1["npos"])
    maps1 = _make_core_inputs(plan, lc1, featT1, F_IN)
    res1 = run_bass_kernel_spmd(nc1, maps1, list(range(NCORES)))
    _RESULTS_LOG["l1"] = res1

    h1 = _host_epilogue(res1, plan, lc1)
    featT2 = np.zeros((Bn, F, NPAD), np.float32)
    for g in range(Bn):
        hh = h1[g]
        hh[N:] = 0.0
        featT2[g] = hh.T
    featT2 = featT2.astype(NPBF)

    nc2 = _compile_layer(plan["T_sched"], F, lc2["npos"])
    maps2 = _make_core_inputs(plan, lc2, featT2, F)
    res2 = run_bass_kernel_spmd(nc2, maps2, list(range(NCORES)))
    _RESULTS_LOG["l2"] = res2

    h2 = _host_epilogue(res2, plan, lc2)
    out = np.zeros((Bn, F, N), np.float32)
    for g in range(Bn):
        out[g] = h2[g][plan["rank"], :].T
    return out
